# revision 6
# baseline (speedup 1.0000x reference)
"""Trainium2 Bass kernel for nn_ControlWhile (dense_cnn, 8 cores).

Reference computation:
    x = conv1x1(x, w_pre) + b_pre
    while mean(|x|) < 3.0:
        x = (conv1x1(tanh(conv1x1(x, w_shared) + b_shared), w_loop) + b_loop) * 10
    out = conv1x1(x, w_shared) + b_shared

Everything between tanh nonlinearities is linear (1x1 convs = channel-mixing
GEMMs), so the whole chain collapses into N+1 affine stages separated by N
tanh applications, where N is the loop trip count:
    t_1 = tanh(A1 @ x + c1)            A1 = Ws@Wpre,      c1 = Ws@b_pre + b_s
    t_i = tanh(Am @ t_{i-1} + cm)      Am = 10*Ws@Wl,     cm = 10*Ws@b_l + b_s
    out = Am @ t_N + cm
The trip count N is data-dependent but pixel-local (1x1 convs), so the host
determines it exactly by iterating the recurrence on a pixel sample (with a
full-tensor fallback when the sampled mean is near the 3.0 threshold).

Device mapping: batch-parallel, 1 image per NeuronCore. Per core the image's
147456 pixels are split into 8 groups of 18432 columns; the 16 (or 3) input
channels of each group are stacked on the partition axis, giving rhs tiles of
[128, cols] and block-diagonal stationary weights [128, 128] (8 copies of the
16x16 channel-mix on the diagonal). One matmul then computes 8 pixel groups
at once, using the full PE array. tanh runs on the Scalar engine (ACT) with
the per-stage bias fused in; the final affine stage's bias-add runs on the
Vector engine; results DMA straight back to DRAM.
"""

import os
import sys

sys.path.insert(0, "/opt/trn_rl_repo")

from contextlib import ExitStack

import numpy as np

import concourse.bass as bass
import concourse.tile as tile
from concourse import bacc, mybir
from concourse.bass_utils import run_bass_kernel_spmd

B, CIN, COUT, H, W = 8, 3, 16, 384, 384
PIX = H * W            # 147456 pixels per image
NGRP = 8               # pixel groups stacked on the partition axis
CPP = PIX // NGRP      # 18432 columns per core
FD = 2048              # free-dim chunk per pipeline step (4 PSUM banks)
NFD = CPP // FD        # 9 chunks
MM_N = 512             # max fp32 matmul free dim (1 PSUM bank)
NCORES = 8
F32 = mybir.dt.float32

# Stashed result of the last run_bass_kernel_spmd call (exec_time_ns,
# profile path, ...) so an external harness can report HW timing.
last_run_results = None
last_n_iters = None


def _compose_stages(w_pre, b_pre, w_loop, b_loop, w_shared, b_shared):
    """Fold the linear segments between tanhs into single affine maps (f64)."""
    ws = w_shared.astype(np.float64)
    a1 = ws @ w_pre.astype(np.float64)
    c1 = ws @ b_pre.astype(np.float64) + b_shared.astype(np.float64)
    am = 10.0 * (ws @ w_loop.astype(np.float64))
    cm = 10.0 * (ws @ b_loop.astype(np.float64)) + b_shared.astype(np.float64)
    return (a1.astype(np.float32), c1.astype(np.float32),
            am.astype(np.float32), cm.astype(np.float32))


def _trip_count_on(v, w_loop, b_loop, w_shared, b_shared, margin, max_iters=10000):
    """Run the while-loop recurrence on columns v [16, M]; return trip count,
    or None if any mean|v| lands within `margin` of the 3.0 threshold."""
    wl = w_loop.astype(np.float32)
    ws = w_shared.astype(np.float32)
    bl = b_loop.astype(np.float32)[:, None]
    bs = b_shared.astype(np.float32)[:, None]
    n = 0
    while n < max_iters:
        m = float(np.mean(np.abs(v)))
        if margin > 0.0 and abs(m - 3.0) < margin:
            return None
        if m >= 3.0:
            return n
        v = np.tanh(ws @ v + bs)
        v = wl @ v + bl
        v = v * np.float32(10.0)
        n += 1
    return n


def _trip_count(x, w_pre, b_pre, w_loop, b_loop, w_shared, b_shared):
    """Loop trip count: exact recurrence on a strided pixel sample; falls back
    to the full tensor if a sampled mean is too close to the threshold."""
    xf = np.ascontiguousarray(x.astype(np.float32).transpose(1, 0, 2, 3)).reshape(CIN, -1)
    stride = max(1, xf.shape[1] // (1 << 17))
    xs = xf[:, ::stride]
    v = w_pre.astype(np.float32) @ xs + b_pre.astype(np.float32)[:, None]
    n = _trip_count_on(v, w_loop, b_loop, w_shared, b_shared, margin=0.10)
    if n is None:  # ambiguous under sampling: decide on the full tensor
        v = w_pre.astype(np.float32) @ xf + b_pre.astype(np.float32)[:, None]
        n = _trip_count_on(v, w_loop, b_loop, w_shared, b_shared, margin=0.0)
    return n


def _blockdiag_lhsT(a, ngrp):
    """a [O, C] -> stationary operand [ngrp*C, ngrp*O] with a.T on the diagonal."""
    o, c = a.shape
    l = np.zeros((ngrp * c, ngrp * o), np.float32)
    for g in range(ngrp):
        l[g * c:(g + 1) * c, g * o:(g + 1) * o] = a.T
    return l


def _build_nc(n_tanh):
    """Bass program: per core, n_tanh+1 matmul stages with tanh between."""
    kin = NGRP * CIN  # 24 partitions for the input stage
    nc = bacc.Bacc("TRN2")
    x_d = nc.declare_dram_parameter("x", [kin, CPP], F32, isOutput=False)
    w1_d = nc.declare_dram_parameter("w1", [kin, 128], F32, isOutput=False)
    wm_d = nc.declare_dram_parameter("wm", [128, 128], F32, isOutput=False)
    b1_d = nc.declare_dram_parameter("b1", [128, 1], F32, isOutput=False)
    bm_d = nc.declare_dram_parameter("bm", [128, 1], F32, isOutput=False)
    out_d = nc.declare_dram_parameter("out", [128, CPP], F32, isOutput=True)

    with tile.TileContext(nc) as tc, ExitStack() as ctx:
        consts = ctx.enter_context(tc.tile_pool(name="consts", bufs=1))
        work = ctx.enter_context(tc.tile_pool(name="work", bufs=4))
        outp = ctx.enter_context(tc.tile_pool(name="outp", bufs=3))
        psum = ctx.enter_context(tc.tile_pool(name="psum", bufs=2, space="PSUM"))

        w1_s = consts.tile([kin, 128], F32)
        nc.sync.dma_start(out=w1_s[:], in_=w1_d[:])
        wm_s = consts.tile([128, 128], F32)
        nc.sync.dma_start(out=wm_s[:], in_=wm_d[:])
        b1_s = consts.tile([128, 1], F32)
        nc.sync.dma_start(out=b1_s[:], in_=b1_d[:])
        bm_s = consts.tile([128, 1], F32)
        nc.sync.dma_start(out=bm_s[:], in_=bm_d[:])

        x_s = consts.tile([kin, CPP], F32)
        for j in range(NFD):
            nc.sync.dma_start(out=x_s[:, j * FD:(j + 1) * FD],
                              in_=x_d[:, j * FD:(j + 1) * FD])

        for ci in range(NFD):
            cur = x_s[:, ci * FD:(ci + 1) * FD]
            lhsT = w1_s
            for s in range(n_tanh + 1):
                pt = psum.tile([128, FD], F32, tag="pt")
                for j in range(FD // MM_N):
                    nc.tensor.matmul(
                        pt[:, j * MM_N:(j + 1) * MM_N],
                        lhsT[:],
                        cur[:, j * MM_N:(j + 1) * MM_N],
                        start=True, stop=True,
                    )
                bias = b1_s if s == 0 else bm_s
                if s < n_tanh:
                    nxt = work.tile([128, FD], F32, tag="t")
                    nc.scalar.activation(
                        out=nxt[:], in_=pt[:],
                        func=mybir.ActivationFunctionType.Tanh,
                        bias=bias[:], scale=1.0,
                    )
                    cur, lhsT = nxt, wm_s
                else:
                    ot = outp.tile([128, FD], F32, tag="o")
                    nc.vector.tensor_scalar_add(ot[:], pt[:], bias[:])
                    nc.sync.dma_start(out=out_d[:, ci * FD:(ci + 1) * FD], in_=ot[:])
    nc.compile()  # bacc legalization (splits multi-waits into event semaphores)
    return nc


def _pack_x(xb):
    """[CIN, H, W] -> [NGRP*CIN, CPP]: partition g*CIN+c holds channel c of
    pixel group g."""
    return np.ascontiguousarray(
        xb.reshape(CIN, NGRP, CPP).transpose(1, 0, 2)
    ).reshape(NGRP * CIN, CPP)


def _unpack_out(o):
    """[128, CPP] (partition g*COUT+o) -> [COUT, H, W]."""
    return np.ascontiguousarray(
        o.reshape(NGRP, COUT, CPP).transpose(1, 0, 2)
    ).reshape(COUT, H, W)


def kernel(x, w_pre, b_pre, w_loop, b_loop, w_shared, b_shared):
    global last_run_results, last_n_iters
    x = np.asarray(x, np.float32)
    w_pre = np.asarray(w_pre, np.float32)
    b_pre = np.asarray(b_pre, np.float32)
    w_loop = np.asarray(w_loop, np.float32)
    b_loop = np.asarray(b_loop, np.float32)
    w_shared = np.asarray(w_shared, np.float32)
    b_shared = np.asarray(b_shared, np.float32)

    n = _trip_count(x, w_pre, b_pre, w_loop, b_loop, w_shared, b_shared)
    last_n_iters = n
    a1, c1, am, cm = _compose_stages(w_pre, b_pre, w_loop, b_loop, w_shared, b_shared)

    w1 = _blockdiag_lhsT(a1, NGRP)                       # [24, 128]
    wm = _blockdiag_lhsT(am, NGRP)                       # [128, 128]
    b1 = np.tile(c1, NGRP).astype(np.float32)[:, None]   # [128, 1]
    bm = np.tile(cm, NGRP).astype(np.float32)[:, None]

    nc = _build_nc(n)
    in_maps = [
        {"x": _pack_x(x[i]), "w1": w1, "wm": wm, "b1": b1, "bm": bm}
        for i in range(NCORES)
    ]
    res = run_bass_kernel_spmd(nc, in_maps, list(range(NCORES)))
    last_run_results = res
    return np.stack([_unpack_out(res.results[i]["out"]) for i in range(NCORES)])


# revision 11
# speedup vs baseline: 1.6243x; 1.6243x over previous
"""Trainium2 Bass kernel for nn_ControlWhile (dense_cnn, 8 cores).

Reference computation:
    x = conv1x1(x, w_pre) + b_pre
    while mean(|x|) < 3.0:
        x = (conv1x1(tanh(conv1x1(x, w_shared) + b_shared), w_loop) + b_loop) * 10
    out = conv1x1(x, w_shared) + b_shared

Everything between tanh nonlinearities is linear (1x1 convs = channel-mixing
GEMMs), so the whole chain collapses into N+1 affine stages separated by N
tanh applications, where N is the loop trip count:
    t_1 = tanh(A1 @ x + c1)            A1 = Ws@Wpre,      c1 = Ws@b_pre + b_s
    t_i = tanh(Am @ t_{i-1} + cm)      Am = 10*Ws@Wl,     cm = 10*Ws@b_l + b_s
    out = Am @ t_N + cm
The trip count N is data-dependent but pixel-local (1x1 convs), so the host
determines it exactly by iterating the recurrence on a pixel sample (with a
full-tensor fallback when the sampled mean is near the 3.0 threshold).

Device mapping: batch-parallel, 1 image per NeuronCore. Per core the image's
147456 pixels are split into 8 groups of 18432 columns; the 16 (or 3) input
channels of each group are stacked on the partition axis, giving rhs tiles of
[128, cols] and block-diagonal stationary weights [128, 128] (8 copies of the
16x16 channel-mix on the diagonal). One matmul then computes 8 pixel groups
at once, using the full PE array. tanh runs on the Scalar engine (ACT) with
the per-stage bias fused in; the final affine stage's bias-add runs on the
Vector engine; results DMA straight back to DRAM.
"""

import os
import sys

sys.path.insert(0, "/opt/trn_rl_repo")

from contextlib import ExitStack

import numpy as np

import concourse.bass as bass
import concourse.tile as tile
from concourse import bacc, mybir
from concourse.bass_utils import run_bass_kernel_spmd

B, CIN, COUT, H, W = 8, 3, 16, 384, 384
PIX = H * W            # 147456 pixels per image
NGRP = 8               # pixel groups stacked on the partition axis
CPP = PIX // NGRP      # 18432 columns per core
FD = 2048              # free-dim chunk per pipeline step (4 PSUM banks)
NFD = CPP // FD        # 9 chunks
MM_N = 512             # max fp32 matmul free dim (1 PSUM bank)
NCORES = 8
F32 = mybir.dt.float32
F32R = mybir.dt.float32r  # fp32 bytes, single-pass PE mode (1 cyc/row at N>=256)

# Stashed result of the last run_bass_kernel_spmd call (exec_time_ns,
# profile path, ...) so an external harness can report HW timing.
last_run_results = None
last_n_iters = None


def _compose_stages(w_pre, b_pre, w_loop, b_loop, w_shared, b_shared):
    """Fold the linear segments between tanhs into single affine maps (f64)."""
    ws = w_shared.astype(np.float64)
    a1 = ws @ w_pre.astype(np.float64)
    c1 = ws @ b_pre.astype(np.float64) + b_shared.astype(np.float64)
    am = 10.0 * (ws @ w_loop.astype(np.float64))
    cm = 10.0 * (ws @ b_loop.astype(np.float64)) + b_shared.astype(np.float64)
    return (a1.astype(np.float32), c1.astype(np.float32),
            am.astype(np.float32), cm.astype(np.float32))


def _trip_count_on(v, w_loop, b_loop, w_shared, b_shared, margin, max_iters=10000):
    """Run the while-loop recurrence on columns v [16, M]; return trip count,
    or None if any mean|v| lands within `margin` of the 3.0 threshold."""
    wl = w_loop.astype(np.float32)
    ws = w_shared.astype(np.float32)
    bl = b_loop.astype(np.float32)[:, None]
    bs = b_shared.astype(np.float32)[:, None]
    n = 0
    while n < max_iters:
        m = float(np.mean(np.abs(v)))
        if margin > 0.0 and abs(m - 3.0) < margin:
            return None
        if m >= 3.0:
            return n
        v = np.tanh(ws @ v + bs)
        v = wl @ v + bl
        v = v * np.float32(10.0)
        n += 1
    return n


def _trip_count(x, w_pre, b_pre, w_loop, b_loop, w_shared, b_shared):
    """Loop trip count: exact recurrence on a strided pixel sample; falls back
    to the full tensor if a sampled mean is too close to the threshold."""
    xf = np.ascontiguousarray(x.astype(np.float32).transpose(1, 0, 2, 3)).reshape(CIN, -1)
    stride = max(1, xf.shape[1] // (1 << 17))
    xs = xf[:, ::stride]
    v = w_pre.astype(np.float32) @ xs + b_pre.astype(np.float32)[:, None]
    n = _trip_count_on(v, w_loop, b_loop, w_shared, b_shared, margin=0.10)
    if n is None:  # ambiguous under sampling: decide on the full tensor
        v = w_pre.astype(np.float32) @ xf + b_pre.astype(np.float32)[:, None]
        n = _trip_count_on(v, w_loop, b_loop, w_shared, b_shared, margin=0.0)
    return n


def _blockdiag_lhsT(a, ngrp):
    """a [O, C] -> stationary operand [ngrp*C, ngrp*O] with a.T on the diagonal."""
    o, c = a.shape
    l = np.zeros((ngrp * c, ngrp * o), np.float32)
    for g in range(ngrp):
        l[g * c:(g + 1) * c, g * o:(g + 1) * o] = a.T
    return l


def _build_nc(n_tanh):
    """Bass program: per core, n_tanh+1 matmul stages with tanh between."""
    kin = NGRP * CIN  # 24 partitions for the input stage
    nc = bacc.Bacc("TRN2")
    x_d = nc.declare_dram_parameter("x", [kin, CPP], F32R, isOutput=False)
    w1_d = nc.declare_dram_parameter("w1", [kin, 128], F32R, isOutput=False)
    wm_d = nc.declare_dram_parameter("wm", [128, 128], F32R, isOutput=False)
    b1_d = nc.declare_dram_parameter("b1", [128, 1], F32, isOutput=False)
    bm_d = nc.declare_dram_parameter("bm", [128, 1], F32, isOutput=False)
    out_d = nc.declare_dram_parameter("out", [128, CPP], F32, isOutput=True)

    with tile.TileContext(nc) as tc, ExitStack() as ctx:
        consts = ctx.enter_context(tc.tile_pool(name="consts", bufs=1))
        work = ctx.enter_context(tc.tile_pool(name="work", bufs=4))
        outp = ctx.enter_context(tc.tile_pool(name="outp", bufs=3))
        psum = ctx.enter_context(tc.tile_pool(name="psum", bufs=2, space="PSUM"))

        w1_s = consts.tile([kin, 128], F32R)
        nc.sync.dma_start(out=w1_s[:], in_=w1_d[:])
        wm_s = consts.tile([128, 128], F32R)
        nc.sync.dma_start(out=wm_s[:], in_=wm_d[:])
        b1_s = consts.tile([128, 1], F32)
        nc.sync.dma_start(out=b1_s[:], in_=b1_d[:])
        bm_s = consts.tile([128, 1], F32)
        nc.sync.dma_start(out=bm_s[:], in_=bm_d[:])

        x_s = consts.tile([kin, CPP], F32R)
        for j in range(NFD):
            nc.sync.dma_start(out=x_s[:, j * FD:(j + 1) * FD],
                              in_=x_d[:, j * FD:(j + 1) * FD])

        for ci in range(NFD):
            cur = x_s[:, ci * FD:(ci + 1) * FD]
            lhsT = w1_s
            for s in range(n_tanh + 1):
                pt = psum.tile([128, FD], F32, tag="pt")
                for j in range(FD // MM_N):
                    nc.tensor.matmul(
                        pt[:, j * MM_N:(j + 1) * MM_N],
                        lhsT[:],
                        cur[:, j * MM_N:(j + 1) * MM_N],
                        start=True, stop=True,
                    )
                bias = b1_s if s == 0 else bm_s
                if s < n_tanh:
                    nxt = work.tile([128, FD], F32R, tag="t")
                    nc.scalar.activation(
                        out=nxt[:], in_=pt[:],
                        func=mybir.ActivationFunctionType.Tanh,
                        bias=bias[:], scale=1.0,
                    )
                    cur, lhsT = nxt, wm_s
                else:
                    ot = outp.tile([128, FD], F32, tag="o")
                    nc.vector.tensor_scalar_add(ot[:], pt[:], bias[:])
                    nc.sync.dma_start(out=out_d[:, ci * FD:(ci + 1) * FD], in_=ot[:])
    nc.compile()  # bacc legalization (splits multi-waits into event semaphores)
    return nc


def _pack_x(xb):
    """[CIN, H, W] -> [NGRP*CIN, CPP]: partition g*CIN+c holds channel c of
    pixel group g."""
    return np.ascontiguousarray(
        xb.reshape(CIN, NGRP, CPP).transpose(1, 0, 2)
    ).reshape(NGRP * CIN, CPP)


def _unpack_out(o):
    """[128, CPP] (partition g*COUT+o) -> [COUT, H, W]."""
    return np.ascontiguousarray(
        o.reshape(NGRP, COUT, CPP).transpose(1, 0, 2)
    ).reshape(COUT, H, W)


def kernel(x, w_pre, b_pre, w_loop, b_loop, w_shared, b_shared):
    global last_run_results, last_n_iters
    x = np.asarray(x, np.float32)
    w_pre = np.asarray(w_pre, np.float32)
    b_pre = np.asarray(b_pre, np.float32)
    w_loop = np.asarray(w_loop, np.float32)
    b_loop = np.asarray(b_loop, np.float32)
    w_shared = np.asarray(w_shared, np.float32)
    b_shared = np.asarray(b_shared, np.float32)

    n = _trip_count(x, w_pre, b_pre, w_loop, b_loop, w_shared, b_shared)
    last_n_iters = n
    a1, c1, am, cm = _compose_stages(w_pre, b_pre, w_loop, b_loop, w_shared, b_shared)

    w1 = _blockdiag_lhsT(a1, NGRP)                       # [24, 128]
    wm = _blockdiag_lhsT(am, NGRP)                       # [128, 128]
    b1 = np.tile(c1, NGRP).astype(np.float32)[:, None]   # [128, 1]
    bm = np.tile(cm, NGRP).astype(np.float32)[:, None]

    nc = _build_nc(n)
    in_maps = [
        {"x": _pack_x(x[i]), "w1": w1, "wm": wm, "b1": b1, "bm": bm}
        for i in range(NCORES)
    ]
    res = run_bass_kernel_spmd(nc, in_maps, list(range(NCORES)))
    last_run_results = res
    return np.stack([_unpack_out(res.results[i]["out"]) for i in range(NCORES)])


# revision 17
# speedup vs baseline: 1.7219x; 1.0601x over previous
"""Trainium2 Bass kernel for nn_ControlWhile (dense_cnn, 8 cores).

Reference computation:
    x = conv1x1(x, w_pre) + b_pre
    while mean(|x|) < 3.0:
        x = (conv1x1(tanh(conv1x1(x, w_shared) + b_shared), w_loop) + b_loop) * 10
    out = conv1x1(x, w_shared) + b_shared

Everything between tanh nonlinearities is linear (1x1 convs = channel-mixing
GEMMs), so the whole chain collapses into N+1 affine stages separated by N
tanh applications, where N is the loop trip count:
    t_1 = tanh(A1 @ x + c1)            A1 = Ws@Wpre,      c1 = Ws@b_pre + b_s
    t_i = tanh(Am @ t_{i-1} + cm)      Am = 10*Ws@Wl,     cm = 10*Ws@b_l + b_s
    out = Am @ t_N + cm
The trip count N is data-dependent but pixel-local (1x1 convs), so the host
determines it exactly by iterating the recurrence on a pixel sample (with a
full-tensor fallback when the sampled mean is near the 3.0 threshold).

Device mapping: batch-parallel, 1 image per NeuronCore. Per core the image's
147456 pixels are split into 8 groups of 18432 columns; the 16 (or 3) input
channels of each group are stacked on the partition axis, giving rhs tiles of
[128, cols] and block-diagonal stationary weights [128, 128] (8 copies of the
16x16 channel-mix on the diagonal). One matmul then computes 8 pixel groups
at once, using the full PE array. tanh runs on the Scalar engine (ACT) with
the per-stage bias fused in; the final affine stage's bias-add runs on the
Vector engine; results DMA straight back to DRAM.
"""

import os
import sys

sys.path.insert(0, "/opt/trn_rl_repo")

from contextlib import ExitStack

import numpy as np

import concourse.bass as bass
import concourse.tile as tile
from concourse import bacc, mybir
from concourse.bass_utils import run_bass_kernel_spmd

B, CIN, COUT, H, W = 8, 3, 16, 384, 384
PIX = H * W            # 147456 pixels per image
NGRP = 8               # pixel groups stacked on the partition axis
CPP = PIX // NGRP      # 18432 columns per core
FD = 2048              # free-dim chunk per pipeline step (4 PSUM banks)
NFD = CPP // FD        # 9 chunks
MM_N = 512             # max fp32 matmul free dim (1 PSUM bank)
NCORES = 8
F32 = mybir.dt.float32
F16 = mybir.dt.float16  # 1 cyc/row on PE + fast weight load; fp32 PSUM accumulate

# Stashed result of the last run_bass_kernel_spmd call (exec_time_ns,
# profile path, ...) so an external harness can report HW timing.
last_run_results = None
last_n_iters = None


def _compose_stages(w_pre, b_pre, w_loop, b_loop, w_shared, b_shared):
    """Fold the linear segments between tanhs into single affine maps (f64)."""
    ws = w_shared.astype(np.float64)
    a1 = ws @ w_pre.astype(np.float64)
    c1 = ws @ b_pre.astype(np.float64) + b_shared.astype(np.float64)
    am = 10.0 * (ws @ w_loop.astype(np.float64))
    cm = 10.0 * (ws @ b_loop.astype(np.float64)) + b_shared.astype(np.float64)
    return (a1.astype(np.float32), c1.astype(np.float32),
            am.astype(np.float32), cm.astype(np.float32))


def _trip_count_on(v, w_loop, b_loop, w_shared, b_shared, margin, max_iters=10000):
    """Run the while-loop recurrence on columns v [16, M]; return trip count,
    or None if any mean|v| lands within `margin` of the 3.0 threshold."""
    wl = w_loop.astype(np.float32)
    ws = w_shared.astype(np.float32)
    bl = b_loop.astype(np.float32)[:, None]
    bs = b_shared.astype(np.float32)[:, None]
    n = 0
    while n < max_iters:
        m = float(np.mean(np.abs(v)))
        if margin > 0.0 and abs(m - 3.0) < margin:
            return None
        if m >= 3.0:
            return n
        v = np.tanh(ws @ v + bs)
        v = wl @ v + bl
        v = v * np.float32(10.0)
        n += 1
    return n


def _trip_count(x, w_pre, b_pre, w_loop, b_loop, w_shared, b_shared):
    """Loop trip count: exact recurrence on a strided pixel sample; falls back
    to the full tensor if a sampled mean is too close to the threshold."""
    xf = np.ascontiguousarray(x.astype(np.float32).transpose(1, 0, 2, 3)).reshape(CIN, -1)
    stride = max(1, xf.shape[1] // (1 << 17))
    xs = xf[:, ::stride]
    v = w_pre.astype(np.float32) @ xs + b_pre.astype(np.float32)[:, None]
    n = _trip_count_on(v, w_loop, b_loop, w_shared, b_shared, margin=0.10)
    if n is None:  # ambiguous under sampling: decide on the full tensor
        v = w_pre.astype(np.float32) @ xf + b_pre.astype(np.float32)[:, None]
        n = _trip_count_on(v, w_loop, b_loop, w_shared, b_shared, margin=0.0)
    return n


def _blockdiag_lhsT(a, ngrp):
    """a [O, C] -> stationary operand [ngrp*C, ngrp*O] with a.T on the diagonal."""
    o, c = a.shape
    l = np.zeros((ngrp * c, ngrp * o), np.float32)
    for g in range(ngrp):
        l[g * c:(g + 1) * c, g * o:(g + 1) * o] = a.T
    return l


def _build_nc(n_tanh):
    """Bass program: per core, n_tanh+1 matmul stages with tanh between."""
    kin = NGRP * CIN  # 24 partitions for the input stage
    nc = bacc.Bacc("TRN2")
    x_d = nc.declare_dram_parameter("x", [kin, CPP], F16, isOutput=False)
    w1_d = nc.declare_dram_parameter("w1", [kin, 128], F16, isOutput=False)
    wm_d = nc.declare_dram_parameter("wm", [128, 128], F16, isOutput=False)
    b1_d = nc.declare_dram_parameter("b1", [128, 1], F32, isOutput=False)
    bm_d = nc.declare_dram_parameter("bm", [128, 1], F32, isOutput=False)
    out_d = nc.declare_dram_parameter("out", [128, CPP], F32, isOutput=True)

    with tile.TileContext(nc) as tc, ExitStack() as ctx:
        consts = ctx.enter_context(tc.tile_pool(name="consts", bufs=1))
        work = ctx.enter_context(tc.tile_pool(name="work", bufs=4))
        outp = ctx.enter_context(tc.tile_pool(name="outp", bufs=3))
        psum = ctx.enter_context(tc.tile_pool(name="psum", bufs=2, space="PSUM"))

        w1_s = consts.tile([kin, 128], F16)
        nc.sync.dma_start(out=w1_s[:], in_=w1_d[:])
        wm_s = consts.tile([128, 128], F16)
        nc.sync.dma_start(out=wm_s[:], in_=wm_d[:])
        b1_s = consts.tile([128, 1], F32)
        nc.sync.dma_start(out=b1_s[:], in_=b1_d[:])
        bm_s = consts.tile([128, 1], F32)
        nc.sync.dma_start(out=bm_s[:], in_=bm_d[:])

        x_s = consts.tile([kin, CPP], F16)
        for j in range(NFD):
            nc.sync.dma_start(out=x_s[:, j * FD:(j + 1) * FD],
                              in_=x_d[:, j * FD:(j + 1) * FD])

        for ci in range(NFD):
            cur = x_s[:, ci * FD:(ci + 1) * FD]
            lhsT = w1_s
            for s in range(n_tanh + 1):
                pt = psum.tile([128, FD], F32, tag="pt")
                for j in range(FD // MM_N):
                    nc.tensor.matmul(
                        pt[:, j * MM_N:(j + 1) * MM_N],
                        lhsT[:],
                        cur[:, j * MM_N:(j + 1) * MM_N],
                        start=True, stop=True,
                    )
                bias = b1_s if s == 0 else bm_s
                if s < n_tanh:
                    nxt = work.tile([128, FD], F16, tag="t")
                    nc.scalar.activation(
                        out=nxt[:], in_=pt[:],
                        func=mybir.ActivationFunctionType.Tanh,
                        bias=bias[:], scale=1.0,
                    )
                    cur, lhsT = nxt, wm_s
                else:
                    ot = outp.tile([128, FD], F32, tag="o")
                    nc.vector.tensor_scalar_add(ot[:], pt[:], bias[:])
                    nc.sync.dma_start(out=out_d[:, ci * FD:(ci + 1) * FD], in_=ot[:])
    nc.compile()  # bacc legalization (splits multi-waits into event semaphores)
    return nc


def _pack_x(xb):
    """[CIN, H, W] -> [NGRP*CIN, CPP]: partition g*CIN+c holds channel c of
    pixel group g."""
    return np.ascontiguousarray(
        xb.reshape(CIN, NGRP, CPP).transpose(1, 0, 2)
    ).reshape(NGRP * CIN, CPP)


def _unpack_out(o):
    """[128, CPP] (partition g*COUT+o) -> [COUT, H, W]."""
    return np.ascontiguousarray(
        o.reshape(NGRP, COUT, CPP).transpose(1, 0, 2)
    ).reshape(COUT, H, W)


def kernel(x, w_pre, b_pre, w_loop, b_loop, w_shared, b_shared):
    global last_run_results, last_n_iters
    x = np.asarray(x, np.float32)
    w_pre = np.asarray(w_pre, np.float32)
    b_pre = np.asarray(b_pre, np.float32)
    w_loop = np.asarray(w_loop, np.float32)
    b_loop = np.asarray(b_loop, np.float32)
    w_shared = np.asarray(w_shared, np.float32)
    b_shared = np.asarray(b_shared, np.float32)

    n = _trip_count(x, w_pre, b_pre, w_loop, b_loop, w_shared, b_shared)
    last_n_iters = n
    a1, c1, am, cm = _compose_stages(w_pre, b_pre, w_loop, b_loop, w_shared, b_shared)

    w1 = _blockdiag_lhsT(a1, NGRP)                       # [24, 128]
    wm = _blockdiag_lhsT(am, NGRP)                       # [128, 128]
    b1 = np.tile(c1, NGRP).astype(np.float32)[:, None]   # [128, 1]
    bm = np.tile(cm, NGRP).astype(np.float32)[:, None]

    nc = _build_nc(n)
    in_maps = [
        {"x": _pack_x(x[i]).astype(np.float16), "w1": w1.astype(np.float16),
         "wm": wm.astype(np.float16), "b1": b1, "bm": bm}
        for i in range(NCORES)
    ]
    res = run_bass_kernel_spmd(nc, in_maps, list(range(NCORES)))
    last_run_results = res
    return np.stack([_unpack_out(res.results[i]["out"]) for i in range(NCORES)])
